# revision 1
# baseline (speedup 1.0000x reference)
"""MoE grouped-experts kernel for Trainium2 (8 NeuronCores, expert-parallel).

Strategy
--------
Expert-parallel: 32 experts packed onto 8 cores x 4 slots. Routing
(sort-by-expert, capacity truncation at the reference's C=1024) is computed
on host from the tiny `indices` tensor; token rows are gathered per expert,
padded to the slot capacity, and pre-transposed so the device kernel is a
pure stream of fp32r matmuls with zero on-device transposes:

  GEMM1 (h^T orientation):  hT[m,c] = sum_k gup[k,m] * xT[k,c]
        stationary = gup tile [128 D, 128 cols-of-2I], moving = xT tokens
  act:  aT = silu(1.702*min(gate,7)) * (clip(up,-7,7)+1)   (the 1/1.702 is
        folded into the routing probs applied at GEMM2 eviction)
  GEMM2: y[c,d] = sum_k aT[k,c] * down[k,d], eviction scaled by probs/1.702.

Slot capacities adapt to the actual expert loads (same structure on every
core - SPMD): slot j's capacity = max token-block count among the experts
assigned to slot j across cores. All matmuls run as float32r (single-pass,
full PE rate at moving dim >= 256) with fp32 PSUM accumulation; accumulation
groups are interleaved in pairs so the 4-byte weight loads hide under the
previous matmul's streaming.
"""

import math
from contextlib import ExitStack

import numpy as np

N_TOKENS, DIM = 4096, 2048
N_EXPERTS, TOPK, INTER = 32, 4, 1408
ALPHA, LIMIT, LIN_OFFSET = 1.702, 7.0, 1.0

NCORE = 8
NSLOT = N_EXPERTS // NCORE        # expert slots per core = 4
KD = DIM // 128                   # 16 contraction tiles for GEMM1
KI = INTER // 128                 # 11 contraction tiles for GEMM2
DW = 512                          # GEMM2 moving-dim chunk over DIM
NDC = DIM // DW                   # 4
C_REF = 2 * ((N_TOKENS * TOPK + N_EXPERTS - 1) // N_EXPERTS)  # 1024

_PROG_CACHE: dict = {}


def _token_groups(cap: int):
    """Split cap into moving-dim groups, each <= 512 and >= 256 (fp32r full rate)."""
    ng = max(1, math.ceil(cap / 512))
    base = cap // ng
    sizes = [base] * ng
    for i in range(cap - base * ng):
        sizes[i] += 1
    out, off = [], 0
    for s in sizes:
        out.append((off, s))
        off += s
    return out


def _build_program(caps: tuple):
    import concourse.bacc as bacc
    import concourse.mybir as mybir
    import concourse.tile as tile
    from concourse.alu_op_type import AluOpType

    F32 = mybir.dt.float32
    F32R = mybir.dt.float32r
    SB = sum(caps)                      # total 128-row blocks per core
    cmax = max(caps) * 128
    xt_sizes = [128 * KD * c * 128 for c in caps]
    xt_off = np.concatenate([[0], np.cumsum(xt_sizes)]).tolist()
    soff = np.concatenate([[0], np.cumsum(caps)]).tolist()  # block offsets

    nc = bacc.Bacc(None, target_bir_lowering=False, debug=False)
    with ExitStack() as ctx:
        tc = ctx.enter_context(tile.TileContext(nc))
        dram = ctx.enter_context(tc.tile_pool(name="dram", bufs=1, space="DRAM"))
        xt_d = dram.tile([xt_off[-1]], F32R, kind="ExternalInput")
        gup_d = dram.tile([NSLOT, 2, KI, 128, KD * 128], F32R, kind="ExternalInput")
        down_d = dram.tile([NSLOT, NDC, 128, KI * DW], F32R, kind="ExternalInput")
        probs_d = dram.tile([128, SB], F32, kind="ExternalInput")
        y_d = dram.tile([SB, 128, DIM], F32, kind="ExternalOutput")
        names = {
            "xt": xt_d.name, "gup": gup_d.name, "down": down_d.name,
            "probs": probs_d.name, "y": y_d.name,
        }

        xt_pool = ctx.enter_context(tc.tile_pool(name="xt", bufs=3))
        gup_pool = ctx.enter_context(tc.tile_pool(name="gup", bufs=5))
        down_pool = ctx.enter_context(tc.tile_pool(name="down", bufs=2))
        at_pool = ctx.enter_context(tc.tile_pool(name="at", bufs=1))
        fg_pool = ctx.enter_context(tc.tile_pool(name="fg", bufs=3))
        tmp_pool = ctx.enter_context(tc.tile_pool(name="tmp", bufs=4))
        y_pool = ctx.enter_context(tc.tile_pool(name="yt", bufs=3))
        pr_pool = ctx.enter_context(tc.tile_pool(name="pr", bufs=1))
        psg1 = ctx.enter_context(tc.tile_pool(name="psg1", bufs=4, space="PSUM"))
        psg2 = ctx.enter_context(tc.tile_pool(name="psg2", bufs=3, space="PSUM"))

        probs_sb = pr_pool.tile([128, SB], F32)
        nc.sync.dma_start(out=probs_sb[:], in_=probs_d[:])

        for j in range(NSLOT):
            CAP = caps[j] * 128
            capb = caps[j]
            groups = _token_groups(CAP)
            half_elems = 128 * (KD // 2) * CAP

            # xT in two half-slabs (k 0..7 / k 8..15) for cheap cross-expert prefetch
            xt_h = []
            for h in (0, 1):
                t = xt_pool.tile([128, (KD // 2) * cmax], F32R, tag="xt")
                src = xt_d[xt_off[j] + h * half_elems: xt_off[j] + (h + 1) * half_elems]
                nc.sync.dma_start(
                    out=t[:, :(KD // 2) * CAP],
                    in_=src.rearrange("(p c) -> p c", p=128),
                )
                xt_h.append(t)

            def xt_ap(k, g0, gw, CAP=CAP, xt_h=xt_h):
                t = xt_h[k // (KD // 2)]
                kk = k % (KD // 2)
                return t[:, kk * CAP + g0: kk * CAP + g0 + gw]

            at_sb = at_pool.tile([128, KI * cmax], F32R, tag="at")

            for i in range(KI):
                for half in (0, 1):  # 0 = gate, 1 = up
                    gsb = gup_pool.tile([128, KD * 128], F32R, tag="gup")
                    nc.sync.dma_start(out=gsb[:], in_=gup_d[j, half, i])
                    pss = [psg1.tile([128, 512], F32, tag="ps1", name=f"ps1_{i}_{half}_{gi}") for gi in range(len(groups))]
                    for k in range(KD):
                        for gi, (g0, gw) in enumerate(groups):
                            nc.tensor.matmul(
                                pss[gi][:, :gw],
                                lhsT=gsb[:, k * 128:(k + 1) * 128],
                                rhs=xt_ap(k, g0, gw),
                                start=(k == 0), stop=(k == KD - 1),
                            )
                    for gi, (g0, gw) in enumerate(groups):
                        ps = pss[gi]
                        if half == 0:
                            t0 = tmp_pool.tile([128, 512], F32, tag="t0")
                            nc.vector.tensor_scalar_min(t0[:, :gw], ps[:, :gw], LIMIT)
                            fg = fg_pool.tile([128, 512], F32, tag="fg")
                            nc.scalar.activation(
                                fg[:, :gw], t0[:, :gw],
                                mybir.ActivationFunctionType.Silu, scale=ALPHA,
                            )
                            if gi == 0:
                                fgs = [fg]
                            else:
                                fgs.append(fg)
                        else:
                            uc = tmp_pool.tile([128, 512], F32, tag="uc")
                            nc.vector.tensor_scalar(
                                uc[:, :gw], ps[:, :gw], LIMIT, -LIMIT,
                                AluOpType.min, AluOpType.max,
                            )
                            # aT = (clip(up)+1) * silu(1.702*min(gate,7))
                            nc.vector.scalar_tensor_tensor(
                                at_sb[:, i * CAP + g0: i * CAP + g0 + gw],
                                uc[:, :gw], LIN_OFFSET, fgs[gi][:, :gw],
                                AluOpType.add, AluOpType.mult,
                            )

            for dc in range(NDC):
                dsb = down_pool.tile([128, KI * DW], F32R, tag="down")
                nc.sync.dma_start(out=dsb[:], in_=down_d[j, dc])
                for b in range(capb):
                    ps2 = psg2.tile([128, DW], F32, tag="ps2", name=f"ps2_{dc}_{b}")
                    for k in range(KI):
                        nc.tensor.matmul(
                            ps2[:],
                            lhsT=at_sb[:, k * CAP + b * 128: k * CAP + (b + 1) * 128],
                            rhs=dsb[:, k * DW:(k + 1) * DW],
                            start=(k == 0), stop=(k == KI - 1),
                        )
                    yt = y_pool.tile([128, DW], F32, tag="yt")
                    nc.scalar.activation(
                        yt[:], ps2[:],
                        mybir.ActivationFunctionType.Copy,
                        scale=probs_sb[:, soff[j] + b: soff[j] + b + 1],
                    )
                    nc.sync.dma_start(
                        out=y_d[soff[j] + b, :, dc * DW:(dc + 1) * DW], in_=yt[:]
                    )
    nc.compile()
    return nc, names


def _route(indices, token_mask, weights):
    """Replicate the reference's permute/capacity semantics on host."""
    idx = np.asarray(indices).astype(np.int64)
    mask = np.asarray(token_mask).astype(bool)
    w = np.asarray(weights).astype(np.float32)
    flat_e = np.where(mask[:, None], idx, -1).ravel()
    w_flat = np.where(flat_e >= 0, w.ravel(), 0.0).astype(np.float32)
    tok = np.repeat(np.arange(N_TOKENS, dtype=np.int64), TOPK)

    per_expert = []  # (flat_ids, token_ids, weights), flat order, capped at C_REF
    for e in range(N_EXPERTS):
        ids = np.nonzero(flat_e == e)[0][:C_REF]
        per_expert.append((ids, tok[ids], w_flat[ids]))
    return per_expert


def _pack_slots(per_expert):
    """Assign experts to (core, slot) with identical slot capacities per core."""
    needs = [max(1, math.ceil(len(t) / 128)) for _, t, _ in per_expert]
    order = sorted(range(N_EXPERTS), key=lambda e: -needs[e])
    assign = np.empty((NCORE, NSLOT), np.int64)
    caps = []
    for j in range(NSLOT):
        col = order[j * NCORE:(j + 1) * NCORE]
        for m in range(NCORE):
            assign[m, j] = col[m]
        caps.append(max(needs[e] for e in col))
    return assign, tuple(caps)


def _prepare_core_inputs(x, per_expert, gup, down, assign, caps):
    x = np.ascontiguousarray(np.asarray(x, dtype=np.float32))
    gup = np.asarray(gup, dtype=np.float32)
    down = np.asarray(down, dtype=np.float32)
    SB = sum(caps)
    soff = np.concatenate([[0], np.cumsum(caps)]).tolist()
    xt_sizes = [128 * KD * c * 128 for c in caps]
    xt_off = np.concatenate([[0], np.cumsum(xt_sizes)]).tolist()

    in_maps = []
    for m in range(NCORE):
        xt_buf = np.zeros(xt_off[-1], np.float32)
        gup_buf = np.empty((NSLOT, 2, KI, 128, KD * 128), np.float32)
        down_buf = np.empty((NSLOT, NDC, 128, KI * DW), np.float32)
        probs_buf = np.zeros((128, SB), np.float32)
        for j in range(NSLOT):
            CAP = caps[j] * 128
            e = assign[m, j]
            _, toks, ws = per_expert[e]
            n = len(toks)
            xg = np.zeros((CAP, DIM), np.float32)
            xg[:n] = x[toks]
            # [CAP, KD, 128] -> [128(p), KD, CAP]; store halves contiguously
            xt = xg.reshape(CAP, KD, 128).transpose(2, 1, 0)  # [128, KD, CAP]
            half = KD // 2
            blk = 128 * half * CAP
            xt_buf[xt_off[j]: xt_off[j] + blk] = np.ascontiguousarray(xt[:, :half]).ravel()
            xt_buf[xt_off[j] + blk: xt_off[j] + 2 * blk] = np.ascontiguousarray(xt[:, half:]).ravel()
            pw = np.zeros(CAP, np.float32)
            pw[:n] = ws / ALPHA
            probs_buf[:, soff[j]: soff[j] + caps[j]] = pw.reshape(caps[j], 128).T
            for half_gu in (0, 1):
                hm = gup[e, :, half_gu::2]  # [DIM, INTER] gate or up, deinterleaved
                gup_buf[j, half_gu] = (
                    hm.reshape(KD, 128, KI, 128).transpose(2, 1, 0, 3)
                    .reshape(KI, 128, KD * 128)
                )
            dm = down[e]  # [INTER, DIM]
            down_buf[j] = (
                dm.reshape(KI, 128, NDC, DW).transpose(2, 1, 0, 3)
                .reshape(NDC, 128, KI * DW)
            )
        in_maps.append({
            "xt": xt_buf, "gup": gup_buf, "down": down_buf, "probs": probs_buf,
        })
    return in_maps


def _run(inputs: dict, trace: bool = False, tmpdir=None):
    from concourse.bass_utils import run_bass_kernel_spmd

    x = inputs["x"]
    gup = inputs["gate_and_up_projs"]
    down = inputs["down_projs"]

    per_expert = _route(inputs["indices"], inputs["token_mask"], inputs["weights"])
    assign, caps = _pack_slots(per_expert)

    if caps not in _PROG_CACHE:
        _PROG_CACHE[caps] = _build_program(caps)
    nc, names = _PROG_CACHE[caps]

    core_maps = _prepare_core_inputs(x, per_expert, gup, down, assign, caps)
    in_maps = [{names[k]: v for k, v in mm.items()} for mm in core_maps]
    res = run_bass_kernel_spmd(
        nc, in_maps, list(range(NCORE)), trace=trace, tmpdir=tmpdir,
    )

    SB = sum(caps)
    soff = np.concatenate([[0], np.cumsum(caps)]).tolist()
    # stack y rows core-major; expert (m, j) rows at m*SB*128 + soff[j]*128
    ys = [np.asarray(res.results[m][names["y"]]).reshape(SB * 128, DIM)
          for m in range(NCORE)]
    Y = np.concatenate(ys + [np.zeros((1, DIM), np.float32)], axis=0)

    pos = np.full(N_TOKENS * TOPK, NCORE * SB * 128, np.int64)  # default zeros row
    slot_of = {int(assign[m, j]): (m, j) for m in range(NCORE) for j in range(NSLOT)}
    for e in range(N_EXPERTS):
        ids, _, _ = per_expert[e]
        m, j = slot_of[e]
        pos[ids] = m * SB * 128 + soff[j] * 128 + np.arange(len(ids))

    contrib = Y[pos]  # probs already applied on device
    out = contrib.reshape(N_TOKENS, TOPK, DIM).sum(axis=1, dtype=np.float32)
    return out.astype(np.float32), res


def kernel(**inputs) -> np.ndarray:
    out, _ = _run(inputs, trace=False)
    return out



# revision 2
# speedup vs baseline: 1.2326x; 1.2326x over previous
"""MoE grouped-experts kernel for Trainium2 (8 NeuronCores, expert-parallel).

Strategy
--------
Expert-parallel: 32 experts packed onto 8 cores x 4 slots. Routing
(sort-by-expert, capacity truncation at the reference's C=1024) is computed
on host from the tiny `indices` tensor; token rows are gathered per expert
and pre-transposed so the device kernel is a pure stream of bf16 matmuls
with zero on-device transposes:

  GEMM1 (h^T orientation):  hT[m,c] = sum_k gup[k,m] * xT[k,c]
        stationary = gup tile [128 D, 128 cols-of-2I], moving = xT tokens
  act:  aT = silu(1.702*min(gate,7)) * (clip(up,-7,7)+1)   (the 1/1.702 is
        folded into the routing probs applied at GEMM2 eviction)
  GEMM2 (y^T orientation):  yT[d,c] = sum_k down[k,d] * aT[k,c]
        stationary = down tile [128 I, 128 cols-of-D], moving = aT tokens;
        eviction multiplies by a partition-broadcast probs row (probs/1.702).

All matmul operands are bf16 (same 1 cycle/row PE rate as fp32r but half
the HBM traffic; fp32 PSUM accumulation keeps the error ~2e-3). Slot
capacities are EXACT token counts (max over the 8 cores per slot, ~2%
padding vs ~19% for 128-row blocks): the moving dim of both GEMMs is
tokens, so compute scales with the exact count, not a block roundup.
"""

import math
from contextlib import ExitStack

import numpy as np
import ml_dtypes

BF16 = ml_dtypes.bfloat16

N_TOKENS, DIM = 4096, 2048
N_EXPERTS, TOPK, INTER = 32, 4, 1408
ALPHA, LIMIT, LIN_OFFSET = 1.702, 7.0, 1.0

NCORE = 8
NSLOT = N_EXPERTS // NCORE        # expert slots per core = 4
KD = DIM // 128                   # 16 contraction tiles for GEMM1
KI = INTER // 128                 # 11 contraction tiles for GEMM2
ND = DIM // 128                   # 16 output-partition tiles for GEMM2
C_REF = 2 * ((N_TOKENS * TOPK + N_EXPERTS - 1) // N_EXPERTS)  # 1024

_PROG_CACHE: dict = {}


def _token_groups(cap: int):
    """Split cap tokens into balanced moving-dim groups of <= 512 (PSUM bank)."""
    ng = max(1, math.ceil(cap / 512))
    base = cap // ng
    sizes = [base] * ng
    for i in range(cap - base * ng):
        sizes[i] += 1
    out, off = [], 0
    for s in sizes:
        out.append((off, s))
        off += s
    return out


def _build_program(caps: tuple):
    import concourse.bacc as bacc
    import concourse.mybir as mybir
    import concourse.tile as tile
    from concourse.alu_op_type import AluOpType

    F32 = mybir.dt.float32
    BF = mybir.dt.bfloat16
    SUM = sum(caps)
    cmax = max(caps)
    xt_sizes = [128 * KD * c for c in caps]
    xt_off = np.concatenate([[0], np.cumsum(xt_sizes)]).tolist()
    soff = np.concatenate([[0], np.cumsum(caps)]).tolist()

    nc = bacc.Bacc(None, target_bir_lowering=False, debug=False)
    with ExitStack() as ctx:
        tc = ctx.enter_context(tile.TileContext(nc))
        dram = ctx.enter_context(tc.tile_pool(name="dram", bufs=1, space="DRAM"))
        xt_d = dram.tile([xt_off[-1]], BF, kind="ExternalInput")
        gup_d = dram.tile([NSLOT, 2, KI, 128, KD * 128], BF, kind="ExternalInput")
        down_d = dram.tile([NSLOT, ND, 128, KI * 128], BF, kind="ExternalInput")
        probs_d = dram.tile([128, SUM], F32, kind="ExternalInput")
        y_d = dram.tile([ND, 128, SUM], F32, kind="ExternalOutput")
        names = {
            "xt": xt_d.name, "gup": gup_d.name, "down": down_d.name,
            "probs": probs_d.name, "y": y_d.name,
        }

        xt_pool = ctx.enter_context(tc.tile_pool(name="xt", bufs=4))
        gup_pool = ctx.enter_context(tc.tile_pool(name="gup", bufs=5))
        down_pool = ctx.enter_context(tc.tile_pool(name="down", bufs=4))
        at_pool = ctx.enter_context(tc.tile_pool(name="at", bufs=2))
        fg_pool = ctx.enter_context(tc.tile_pool(name="fg", bufs=4))
        tmp_pool = ctx.enter_context(tc.tile_pool(name="tmp", bufs=4))
        y_pool = ctx.enter_context(tc.tile_pool(name="yt", bufs=3))
        pr_pool = ctx.enter_context(tc.tile_pool(name="pr", bufs=1))
        psg1 = ctx.enter_context(tc.tile_pool(name="psg1", bufs=4, space="PSUM"))
        psg2 = ctx.enter_context(tc.tile_pool(name="psg2", bufs=3, space="PSUM"))

        probs_sb = pr_pool.tile([128, SUM], F32)
        nc.sync.dma_start(out=probs_sb[:], in_=probs_d[:])

        for j in range(NSLOT):
            CAP = caps[j]
            groups = _token_groups(CAP)
            half_elems = 128 * (KD // 2) * CAP

            # xT in two half-slabs (k 0..7 / k 8..15) for cheap prefetch
            xt_h = []
            for h in (0, 1):
                t = xt_pool.tile([128, (KD // 2) * cmax], BF, tag="xt")
                src = xt_d[xt_off[j] + h * half_elems: xt_off[j] + (h + 1) * half_elems]
                nc.sync.dma_start(
                    out=t[:, :(KD // 2) * CAP],
                    in_=src.rearrange("(p c) -> p c", p=128),
                )
                xt_h.append(t)

            def xt_ap(k, g0, gw, CAP=CAP, xt_h=xt_h):
                t = xt_h[k // (KD // 2)]
                kk = k % (KD // 2)
                return t[:, kk * CAP + g0: kk * CAP + g0 + gw]

            at_sb = at_pool.tile([128, KI * cmax], BF, tag="at")

            for i in range(KI):
                fgs = []
                for half in (0, 1):  # 0 = gate, 1 = up
                    gsb = gup_pool.tile([128, KD * 128], BF, tag="gup")
                    nc.sync.dma_start(out=gsb[:], in_=gup_d[j, half, i])
                    pss = [psg1.tile([128, 512], F32, tag="ps1",
                                     name=f"ps1_{j}_{i}_{half}_{gi}")
                           for gi in range(len(groups))]
                    for k in range(KD):
                        for gi, (g0, gw) in enumerate(groups):
                            nc.tensor.matmul(
                                pss[gi][:, :gw],
                                lhsT=gsb[:, k * 128:(k + 1) * 128],
                                rhs=xt_ap(k, g0, gw),
                                start=(k == 0), stop=(k == KD - 1),
                            )
                    for gi, (g0, gw) in enumerate(groups):
                        ps = pss[gi]
                        if half == 0:
                            t0 = tmp_pool.tile([128, 512], F32, tag="t0")
                            nc.vector.tensor_scalar_min(t0[:, :gw], ps[:, :gw], LIMIT)
                            fg = fg_pool.tile([128, 512], F32, tag="fg")
                            nc.scalar.activation(
                                fg[:, :gw], t0[:, :gw],
                                mybir.ActivationFunctionType.Silu, scale=ALPHA,
                            )
                            fgs.append(fg)
                        else:
                            uc = tmp_pool.tile([128, 512], F32, tag="uc")
                            nc.vector.tensor_scalar(
                                uc[:, :gw], ps[:, :gw], LIMIT, -LIMIT,
                                AluOpType.min, AluOpType.max,
                            )
                            # aT = (clip(up)+1) * silu(1.702*min(gate,7))
                            nc.vector.scalar_tensor_tensor(
                                at_sb[:, i * CAP + g0: i * CAP + g0 + gw],
                                uc[:, :gw], LIN_OFFSET, fgs[gi][:, :gw],
                                AluOpType.add, AluOpType.mult,
                            )

            # GEMM2, y^T orientation: moving dim = tokens (exact count)
            for t in range(ND):
                dsb = down_pool.tile([128, KI * 128], BF, tag="down")
                nc.sync.dma_start(out=dsb[:], in_=down_d[j, t])
                yt = y_pool.tile([128, cmax], F32, tag="yt")
                for gi, (g0, gw) in enumerate(groups):
                    ps2 = psg2.tile([128, 512], F32, tag="ps2",
                                    name=f"ps2_{j}_{t}_{gi}")
                    for k in range(KI):
                        nc.tensor.matmul(
                            ps2[:, :gw],
                            lhsT=dsb[:, k * 128:(k + 1) * 128],
                            rhs=at_sb[:, k * CAP + g0: k * CAP + g0 + gw],
                            start=(k == 0), stop=(k == KI - 1),
                        )
                    nc.vector.tensor_tensor(
                        yt[:, g0:g0 + gw], ps2[:, :gw],
                        probs_sb[:, soff[j] + g0: soff[j] + g0 + gw],
                        AluOpType.mult,
                    )
                nc.sync.dma_start(
                    out=y_d[t, :, soff[j]:soff[j] + CAP], in_=yt[:, :CAP]
                )
    nc.compile()
    return nc, names


def _route(indices, token_mask, weights):
    """Replicate the reference's permute/capacity semantics on host."""
    idx = np.asarray(indices).astype(np.int64)
    mask = np.asarray(token_mask).astype(bool)
    w = np.asarray(weights).astype(np.float32)
    flat_e = np.where(mask[:, None], idx, -1).ravel()
    w_flat = np.where(flat_e >= 0, w.ravel(), 0.0).astype(np.float32)
    tok = np.repeat(np.arange(N_TOKENS, dtype=np.int64), TOPK)

    per_expert = []  # (flat_ids, token_ids, weights), flat order, capped at C_REF
    for e in range(N_EXPERTS):
        ids = np.nonzero(flat_e == e)[0][:C_REF]
        per_expert.append((ids, tok[ids], w_flat[ids]))
    return per_expert


def _pack_slots(per_expert):
    """Assign experts to (core, slot); slot capacity = max token count."""
    needs = [max(1, len(t)) for _, t, _ in per_expert]
    order = sorted(range(N_EXPERTS), key=lambda e: -needs[e])
    assign = np.empty((NCORE, NSLOT), np.int64)
    caps = []
    for j in range(NSLOT):
        col = order[j * NCORE:(j + 1) * NCORE]
        for m in range(NCORE):
            assign[m, j] = col[m]
        # round up to even so every DMA/AP stays 4-byte aligned in bf16
        caps.append((max(needs[e] for e in col) + 1) // 2 * 2)
    return assign, tuple(caps)


def _prepare_core_inputs(x, per_expert, gup, down, assign, caps):
    x = np.ascontiguousarray(np.asarray(x, dtype=np.float32))
    gup = np.asarray(gup, dtype=np.float32)
    down = np.asarray(down, dtype=np.float32)
    SUM = sum(caps)
    soff = np.concatenate([[0], np.cumsum(caps)]).tolist()
    xt_sizes = [128 * KD * c for c in caps]
    xt_off = np.concatenate([[0], np.cumsum(xt_sizes)]).tolist()

    in_maps = []
    for m in range(NCORE):
        xt_buf = np.zeros(xt_off[-1], BF16)
        gup_buf = np.empty((NSLOT, 2, KI, 128, KD * 128), BF16)
        down_buf = np.empty((NSLOT, ND, 128, KI * 128), BF16)
        probs_buf = np.zeros((128, SUM), np.float32)
        for j in range(NSLOT):
            CAP = caps[j]
            e = assign[m, j]
            _, toks, ws = per_expert[e]
            n = len(toks)
            xg = np.zeros((CAP, KD * 128), BF16)
            xg[:n] = x[toks].astype(BF16)
            # [CAP, KD, 128] -> [128(p), KD, CAP]; store halves contiguously
            xt = xg.reshape(CAP, KD, 128).transpose(2, 1, 0)  # [128, KD, CAP]
            half = KD // 2
            blk = 128 * half * CAP
            xt_buf[xt_off[j]: xt_off[j] + blk] = np.ascontiguousarray(xt[:, :half]).ravel()
            xt_buf[xt_off[j] + blk: xt_off[j] + 2 * blk] = np.ascontiguousarray(xt[:, half:]).ravel()
            pw = np.zeros(CAP, np.float32)
            pw[:n] = ws / ALPHA
            probs_buf[:, soff[j]: soff[j] + CAP] = pw[None, :]
            ge = gup[e].astype(BF16)
            for half_gu in (0, 1):
                hm = ge[:, half_gu::2]  # [DIM, INTER] gate or up, deinterleaved
                gup_buf[j, half_gu] = (
                    hm.reshape(KD, 128, KI, 128).transpose(2, 1, 0, 3)
                    .reshape(KI, 128, KD * 128)
                )
            dm = down[e].astype(BF16)  # [INTER, DIM]
            down_buf[j] = (
                dm.reshape(KI, 128, ND, 128).transpose(2, 1, 0, 3)
                .reshape(ND, 128, KI * 128)
            )
        in_maps.append({
            "xt": xt_buf, "gup": gup_buf, "down": down_buf, "probs": probs_buf,
        })
    return in_maps


def _run(inputs: dict, trace: bool = False, tmpdir=None):
    from concourse.bass_utils import run_bass_kernel_spmd

    x = inputs["x"]
    gup = inputs["gate_and_up_projs"]
    down = inputs["down_projs"]

    per_expert = _route(inputs["indices"], inputs["token_mask"], inputs["weights"])
    assign, caps = _pack_slots(per_expert)

    if caps not in _PROG_CACHE:
        _PROG_CACHE[caps] = _build_program(caps)
    nc, names = _PROG_CACHE[caps]

    core_maps = _prepare_core_inputs(x, per_expert, gup, down, assign, caps)
    in_maps = [{names[k]: v for k, v in mm.items()} for mm in core_maps]
    res = run_bass_kernel_spmd(
        nc, in_maps, list(range(NCORE)), trace=trace, tmpdir=tmpdir,
    )

    SUM = sum(caps)
    soff = np.concatenate([[0], np.cumsum(caps)]).tolist()
    # core m's y is [ND*128, SUM] = yT (dims x padded token columns)
    ys = [np.asarray(res.results[m][names["y"]]).reshape(DIM, SUM)
          for m in range(NCORE)]
    Y = np.concatenate(ys + [np.zeros((DIM, 1), np.float32)], axis=1)

    pos = np.full(N_TOKENS * TOPK, NCORE * SUM, np.int64)  # default zeros col
    slot_of = {int(assign[m, j]): (m, j) for m in range(NCORE) for j in range(NSLOT)}
    for e in range(N_EXPERTS):
        ids, _, _ = per_expert[e]
        m, j = slot_of[e]
        pos[ids] = m * SUM + soff[j] + np.arange(len(ids))

    contrib = Y[:, pos]  # [DIM, N*TOPK]; probs already applied on device
    out = contrib.reshape(DIM, N_TOKENS, TOPK).sum(axis=2, dtype=np.float32).T
    return np.ascontiguousarray(out, dtype=np.float32), res


def kernel(**inputs) -> np.ndarray:
    out, _ = _run(inputs, trace=False)
    return out


# revision 4
# speedup vs baseline: 1.2546x; 1.0178x over previous
"""MoE grouped-experts kernel for Trainium2 (8 NeuronCores, expert-parallel).

Strategy
--------
Expert-parallel: 32 experts packed onto 8 cores x 4 slots. Routing
(sort-by-expert, capacity truncation at the reference's C=1024) is computed
on host from the tiny `indices` tensor; token rows are gathered per expert
and pre-transposed so the device kernel is a pure stream of bf16 matmuls
with zero on-device transposes:

  GEMM1 (h^T orientation):  hT[m,c] = sum_k gup[k,m] * xT[k,c]
        stationary = gup tile [128 D, 128 cols-of-2I], moving = xT tokens
  act:  aT = silu(1.702*min(gate,7)) * (clip(up,-7,7)+1)   (the 1/1.702 is
        folded into the routing probs applied at GEMM2 eviction)
  GEMM2 (y^T orientation):  yT[d,c] = sum_k down[k,d] * aT[k,c]
        stationary = down tile [128 I, 128 cols-of-D], moving = aT tokens;
        eviction multiplies by a partition-broadcast probs row (probs/1.702).

All matmul operands are bf16 (same 1 cycle/row PE rate as fp32r but half
the HBM traffic; fp32 PSUM accumulation keeps the error ~3e-3). Slot
capacities are EXACT token counts (max over the 8 cores per slot, ~2%
padding vs ~19% for 128-row blocks): the moving dim of both GEMMs is
tokens, so compute scales with the exact count, not a block roundup.

Scheduling: GEMM1 of slot j+1 is emitted BEFORE GEMM2 of slot j so the
Tensor queue never waits on the activation chain at a slot boundary.
xt/gup/probs ride the SP HWDGE ring; down/y ride the Activation ring —
two independent FIFOs, so down-prefetch for slot j (emitted ahead of the
silu stream of slot j+1) cannot deadlock against gup loads.
"""

import math
from contextlib import ExitStack

import numpy as np
import ml_dtypes

BF16 = ml_dtypes.bfloat16

N_TOKENS, DIM = 4096, 2048
N_EXPERTS, TOPK, INTER = 32, 4, 1408
ALPHA, LIMIT, LIN_OFFSET = 1.702, 7.0, 1.0

NCORE = 8
NSLOT = N_EXPERTS // NCORE        # expert slots per core = 4
KD = DIM // 128                   # 16 contraction tiles for GEMM1
KI = INTER // 128                 # 11 contraction tiles for GEMM2
ND = DIM // 128                   # 16 output-partition tiles for GEMM2
NQ = 4                            # xT quarter-slabs per slot (KD/4 k-tiles each)
C_REF = 2 * ((N_TOKENS * TOPK + N_EXPERTS - 1) // N_EXPERTS)  # 1024

_PROG_CACHE: dict = {}


def _token_groups(cap: int):
    """Split cap tokens into balanced moving-dim groups of <= 512 (PSUM bank)."""
    ng = max(1, math.ceil(cap / 512))
    base = cap // ng
    sizes = [base] * ng
    for i in range(cap - base * ng):
        sizes[i] += 1
    out, off = [], 0
    for s in sizes:
        out.append((off, s))
        off += s
    return out


def _build_program(caps: tuple):
    import concourse.bacc as bacc
    import concourse.mybir as mybir
    import concourse.tile as tile
    from concourse.alu_op_type import AluOpType

    F32 = mybir.dt.float32
    BF = mybir.dt.bfloat16
    SUM = sum(caps)
    cmax = max(caps)
    KQ = KD // NQ                 # k-tiles per xT quarter-slab = 4
    xt_sizes = [128 * KD * c for c in caps]
    xt_off = np.concatenate([[0], np.cumsum(xt_sizes)]).tolist()
    soff = np.concatenate([[0], np.cumsum(caps)]).tolist()

    nc = bacc.Bacc(None, target_bir_lowering=False, debug=False)
    with ExitStack() as ctx:
        tc = ctx.enter_context(tile.TileContext(nc))
        dram = ctx.enter_context(tc.tile_pool(name="dram", bufs=1, space="DRAM"))
        xt_d = dram.tile([xt_off[-1]], BF, kind="ExternalInput")
        gup_d = dram.tile([NSLOT, 2, KI, 128, KD * 128], BF, kind="ExternalInput")
        down_d = dram.tile([NSLOT, ND, 128, KI * 128], BF, kind="ExternalInput")
        probs_d = dram.tile([128, SUM], F32, kind="ExternalInput")
        y_d = dram.tile([ND, 128, SUM], F32, kind="ExternalOutput")
        names = {
            "xt": xt_d.name, "gup": gup_d.name, "down": down_d.name,
            "probs": probs_d.name, "y": y_d.name,
        }

        xt_pool = ctx.enter_context(tc.tile_pool(name="xt", bufs=2 * NQ))
        gup_pool = ctx.enter_context(tc.tile_pool(name="gup", bufs=5))
        down_pool = ctx.enter_context(tc.tile_pool(name="down", bufs=ND))
        at_pool = ctx.enter_context(tc.tile_pool(name="at", bufs=3))
        fg_pool = ctx.enter_context(tc.tile_pool(name="fg", bufs=4))
        tmp_pool = ctx.enter_context(tc.tile_pool(name="tmp", bufs=4))
        y_pool = ctx.enter_context(tc.tile_pool(name="yt", bufs=3))
        pr_pool = ctx.enter_context(tc.tile_pool(name="pr", bufs=1))
        psg1 = ctx.enter_context(tc.tile_pool(name="psg1", bufs=4, space="PSUM"))
        psg2 = ctx.enter_context(tc.tile_pool(name="psg2", bufs=3, space="PSUM"))

        probs_sb = pr_pool.tile([128, SUM], F32)
        at_tiles = [None] * NSLOT
        down_tiles = [None] * NSLOT

        def emit_down_prefetch(j):
            tiles = []
            for t in range(ND):
                dsb = down_pool.tile([128, KI * 128], BF, tag="down")
                nc.scalar.dma_start(out=dsb[:], in_=down_d[j, t])
                tiles.append(dsb)
            down_tiles[j] = tiles

        def emit_g1(j):
            CAP = caps[j]
            groups = _token_groups(CAP)

            # first gup tile hoisted ahead of the xT slabs: the first matmul
            # needs (i=0,half=0) weights + quarter-slab 0 only
            gsb00 = gup_pool.tile([128, KD * 128], BF, tag="gup")
            nc.sync.dma_start(out=gsb00[:], in_=gup_d[j, 0, 0])

            # xT quarter-slabs (k 0-3 / 4-7 / 8-11 / 12-15)
            q_elems = 128 * KQ * CAP
            xt_q = []
            for h in range(NQ):
                t = xt_pool.tile([128, KQ * cmax], BF, tag="xt")
                src = xt_d[xt_off[j] + h * q_elems: xt_off[j] + (h + 1) * q_elems]
                nc.sync.dma_start(
                    out=t[:, :KQ * CAP],
                    in_=src.rearrange("(p c) -> p c", p=128),
                )
                xt_q.append(t)

            def xt_ap(k, g0, gw):
                t = xt_q[k // KQ]
                kk = k % KQ
                return t[:, kk * CAP + g0: kk * CAP + g0 + gw]

            at_sb = at_pool.tile([128, KI * cmax], BF, tag="at")

            for i in range(KI):
                fgs = []
                for half in (0, 1):  # 0 = gate, 1 = up
                    if i == 0 and half == 0:
                        gsb = gsb00
                    else:
                        gsb = gup_pool.tile([128, KD * 128], BF, tag="gup")
                        nc.sync.dma_start(out=gsb[:], in_=gup_d[j, half, i])
                    pss = [psg1.tile([128, 512], F32, tag="ps1",
                                     name=f"ps1_{j}_{i}_{half}_{gi}")
                           for gi in range(len(groups))]
                    for k in range(KD):
                        for gi, (g0, gw) in enumerate(groups):
                            nc.tensor.matmul(
                                pss[gi][:, :gw],
                                lhsT=gsb[:, k * 128:(k + 1) * 128],
                                rhs=xt_ap(k, g0, gw),
                                start=(k == 0), stop=(k == KD - 1),
                            )
                    for gi, (g0, gw) in enumerate(groups):
                        ps = pss[gi]
                        if half == 0:
                            t0 = tmp_pool.tile([128, 512], F32, tag="t0")
                            nc.vector.tensor_scalar_min(t0[:, :gw], ps[:, :gw], LIMIT)
                            fg = fg_pool.tile([128, 512], F32, tag="fg")
                            nc.scalar.activation(
                                fg[:, :gw], t0[:, :gw],
                                mybir.ActivationFunctionType.Silu, scale=ALPHA,
                            )
                            fgs.append(fg)
                        else:
                            uc = tmp_pool.tile([128, 512], F32, tag="uc")
                            nc.vector.tensor_scalar(
                                uc[:, :gw], ps[:, :gw], LIMIT, -LIMIT,
                                AluOpType.min, AluOpType.max,
                            )
                            # aT = (clip(up)+1) * silu(1.702*min(gate,7))
                            nc.vector.scalar_tensor_tensor(
                                at_sb[:, i * CAP + g0: i * CAP + g0 + gw],
                                uc[:, :gw], LIN_OFFSET, fgs[gi][:, :gw],
                                AluOpType.add, AluOpType.mult,
                            )
            at_tiles[j] = at_sb

        def emit_g2(j, prefetch_down=None):
            CAP = caps[j]
            groups = _token_groups(CAP)
            at_sb = at_tiles[j]
            if prefetch_down is not None:
                down_tiles[prefetch_down] = []
            for t in range(ND):
                if prefetch_down is not None:
                    # interleave next slot's down prefetch so its buffer-reuse
                    # waits resolve tile-by-tile as this slot consumes
                    dnx = down_pool.tile([128, KI * 128], BF, tag="down")
                    nc.scalar.dma_start(out=dnx[:], in_=down_d[prefetch_down, t])
                    down_tiles[prefetch_down].append(dnx)
                dsb = down_tiles[j][t]
                yt = y_pool.tile([128, cmax], F32, tag="yt")
                for gi, (g0, gw) in enumerate(groups):
                    ps2 = psg2.tile([128, 512], F32, tag="ps2",
                                    name=f"ps2_{j}_{t}_{gi}")
                    for k in range(KI):
                        nc.tensor.matmul(
                            ps2[:, :gw],
                            lhsT=dsb[:, k * 128:(k + 1) * 128],
                            rhs=at_sb[:, k * CAP + g0: k * CAP + g0 + gw],
                            start=(k == 0), stop=(k == KI - 1),
                        )
                    nc.vector.tensor_tensor(
                        yt[:, g0:g0 + gw], ps2[:, :gw],
                        probs_sb[:, soff[j] + g0: soff[j] + g0 + gw],
                        AluOpType.mult,
                    )
                nc.scalar.dma_start(
                    out=y_d[t, :, soff[j]:soff[j] + CAP], in_=yt[:, :CAP]
                )

        # Emission schedule: G1(j+1) ahead of G2(j); down(j) prefetched on the
        # Act ring before slot j+1's silu stream; probs after slot 0's loads.
        emit_g1(0)
        nc.sync.dma_start(out=probs_sb[:], in_=probs_d[:])
        emit_down_prefetch(0)
        emit_g1(1)
        emit_g2(0)
        emit_down_prefetch(1)
        emit_g1(2)
        emit_g2(1)
        emit_down_prefetch(2)
        emit_g1(3)
        emit_g2(2, prefetch_down=3)
        emit_g2(3)
    nc.compile()
    return nc, names


def _route(indices, token_mask, weights):
    """Replicate the reference's permute/capacity semantics on host."""
    idx = np.asarray(indices).astype(np.int64)
    mask = np.asarray(token_mask).astype(bool)
    w = np.asarray(weights).astype(np.float32)
    flat_e = np.where(mask[:, None], idx, -1).ravel()
    w_flat = np.where(flat_e >= 0, w.ravel(), 0.0).astype(np.float32)
    tok = np.repeat(np.arange(N_TOKENS, dtype=np.int64), TOPK)

    per_expert = []  # (flat_ids, token_ids, weights), flat order, capped at C_REF
    for e in range(N_EXPERTS):
        ids = np.nonzero(flat_e == e)[0][:C_REF]
        per_expert.append((ids, tok[ids], w_flat[ids]))
    return per_expert


def _pack_slots(per_expert):
    """Assign experts to (core, slot); slot capacity = max token count."""
    needs = [max(1, len(t)) for _, t, _ in per_expert]
    order = sorted(range(N_EXPERTS), key=lambda e: -needs[e])
    assign = np.empty((NCORE, NSLOT), np.int64)
    caps = []
    for j in range(NSLOT):
        col = order[j * NCORE:(j + 1) * NCORE]
        for m in range(NCORE):
            assign[m, j] = col[m]
        # round up to a multiple of 4 so every DMA/AP stays 8-byte aligned
        caps.append(-(-max(needs[e] for e in col) // 4) * 4)
    return assign, tuple(caps)


def _prepare_core_inputs(x, per_expert, gup, down, assign, caps):
    x = np.ascontiguousarray(np.asarray(x, dtype=np.float32))
    gup = np.asarray(gup, dtype=np.float32)
    down = np.asarray(down, dtype=np.float32)
    SUM = sum(caps)
    soff = np.concatenate([[0], np.cumsum(caps)]).tolist()
    xt_sizes = [128 * KD * c for c in caps]
    xt_off = np.concatenate([[0], np.cumsum(xt_sizes)]).tolist()
    KQ = KD // NQ

    in_maps = []
    for m in range(NCORE):
        xt_buf = np.zeros(xt_off[-1], BF16)
        gup_buf = np.empty((NSLOT, 2, KI, 128, KD * 128), BF16)
        down_buf = np.empty((NSLOT, ND, 128, KI * 128), BF16)
        probs_buf = np.zeros((128, SUM), np.float32)
        for j in range(NSLOT):
            CAP = caps[j]
            e = assign[m, j]
            _, toks, ws = per_expert[e]
            n = len(toks)
            xg = np.zeros((CAP, KD * 128), BF16)
            xg[:n] = x[toks].astype(BF16)
            # [CAP, KD, 128] -> [128(p), KD, CAP]; store quarters contiguously
            xt = xg.reshape(CAP, KD, 128).transpose(2, 1, 0)  # [128, KD, CAP]
            blk = 128 * KQ * CAP
            for h in range(NQ):
                xt_buf[xt_off[j] + h * blk: xt_off[j] + (h + 1) * blk] = \
                    np.ascontiguousarray(xt[:, h * KQ:(h + 1) * KQ]).ravel()
            pw = np.zeros(CAP, np.float32)
            pw[:n] = ws / ALPHA
            probs_buf[:, soff[j]: soff[j] + CAP] = pw[None, :]
            ge = gup[e].astype(BF16)
            for half_gu in (0, 1):
                hm = ge[:, half_gu::2]  # [DIM, INTER] gate or up, deinterleaved
                gup_buf[j, half_gu] = (
                    hm.reshape(KD, 128, KI, 128).transpose(2, 1, 0, 3)
                    .reshape(KI, 128, KD * 128)
                )
            dm = down[e].astype(BF16)  # [INTER, DIM]
            down_buf[j] = (
                dm.reshape(KI, 128, ND, 128).transpose(2, 1, 0, 3)
                .reshape(ND, 128, KI * 128)
            )
        in_maps.append({
            "xt": xt_buf, "gup": gup_buf, "down": down_buf, "probs": probs_buf,
        })
    return in_maps


def _run(inputs: dict, trace: bool = False, tmpdir=None):
    from concourse.bass_utils import run_bass_kernel_spmd

    x = inputs["x"]
    gup = inputs["gate_and_up_projs"]
    down = inputs["down_projs"]

    per_expert = _route(inputs["indices"], inputs["token_mask"], inputs["weights"])
    assign, caps = _pack_slots(per_expert)

    if caps not in _PROG_CACHE:
        _PROG_CACHE[caps] = _build_program(caps)
    nc, names = _PROG_CACHE[caps]

    core_maps = _prepare_core_inputs(x, per_expert, gup, down, assign, caps)
    in_maps = [{names[k]: v for k, v in mm.items()} for mm in core_maps]
    res = run_bass_kernel_spmd(
        nc, in_maps, list(range(NCORE)), trace=trace, tmpdir=tmpdir,
    )

    SUM = sum(caps)
    soff = np.concatenate([[0], np.cumsum(caps)]).tolist()
    # core m's y is [ND*128, SUM] = yT (dims x padded token columns)
    ys = [np.asarray(res.results[m][names["y"]]).reshape(DIM, SUM)
          for m in range(NCORE)]
    Y = np.concatenate(ys + [np.zeros((DIM, 1), np.float32)], axis=1)

    pos = np.full(N_TOKENS * TOPK, NCORE * SUM, np.int64)  # default zeros col
    slot_of = {int(assign[m, j]): (m, j) for m in range(NCORE) for j in range(NSLOT)}
    for e in range(N_EXPERTS):
        ids, _, _ = per_expert[e]
        m, j = slot_of[e]
        pos[ids] = m * SUM + soff[j] + np.arange(len(ids))

    contrib = Y[:, pos]  # [DIM, N*TOPK]; probs already applied on device
    out = contrib.reshape(DIM, N_TOKENS, TOPK).sum(axis=2, dtype=np.float32).T
    return np.ascontiguousarray(out, dtype=np.float32), res


def kernel(**inputs) -> np.ndarray:
    out, _ = _run(inputs, trace=False)
    return out
